# revision 36
# baseline (speedup 1.0000x reference)
"""Trainium2 Bass kernel for LorentzBatchNorm.

Math: for points x on the unit hyperboloid (linner(x,x) = -1) and the
normalized centroid `mean` (linner(mean,mean) = -1), the whole module
collapses per point to a rank-1 update:

  alpha = -linner(mean, x)            (one 128-dot per point)
  linner(u,u) = alpha^2 - 1           (u = x - alpha*mean; no 2nd reduction)
  d = arccosh(alpha) = ||x_T||        (Frechet var = mean of d)
  With beta = e0: transport to origin just zeroes channel 0, nu = g*d with
  g = gamma/(var+eps), and

  y[c] = A*x[c] - B*mean[c]  (c >= 1),   y[0] = cosh(nu)
  A = sinh(nu)/sqrt(alpha^2-1)
  B = A * (alpha + u0/(1+mean0)),  u0 = x0 - alpha*mean0

I/O strategy: x is converted to fp16 on the host (rounding ~2.4e-4, well
under the bf16 the previous version used internally) and y is produced as
fp16 on the device and upcast on the host — halving both DMA directions
makes the kernel DMA-bound at ~5.8us/sample (load 2.9 + store 2.9).

Per core (8 samples): points on partitions, channels on free dim,
[128, 32, 128] fp16 per sample; partition p holds points p*32..p*32+31.

Engines per sample (each under the 5.8us DMA budget):
 - PE: 32 per-tile transposes (alpha path), 32 alpha matvecs, and the y
   combine as PSUM matmuls: identity @ t1 (t1 = A*x) accumulated with a
   rank-8 blockdiag matmul negBT-slab @ (I8 (x) mean-row). Plus tiny
   transposes/broadcast matmuls.
 - DVE: t1 = A*x per tile (fp16 4x mode), some transposed-group PSUM->SBUF
   copies (centroid ridden along via accum_out), and the scalar chain.
 - ACT: the yps PSUM -> y_sb fp16 copies and the Sqrt/Ln/Exp chain ops.
 - Pool/GpSimd: remaining transposed-group copies, x0/cosh strided column
   ops, a few [128,32] chain ops.

The program is emitted as a software pipeline: each sample is split into
STAGES and emission is wavefront-ordered (stage = wave - sample), so every
engine's in-order instruction stream interleaves the in-flight samples and
the per-sample dependency chain (~13us deep) overlaps across samples.
"""

import sys

if "/opt/trn_rl_repo" not in sys.path:
    sys.path.insert(0, "/opt/trn_rl_repo")

from contextlib import ExitStack

import numpy as np

import concourse.bass as bass
import concourse.tile as tile
from concourse import mybir
from concourse.vector_clock import ScopedClock

f32 = mybir.dt.float32
f16 = mybir.dt.float16
ALU = mybir.AluOpType
ACTF = mybir.ActivationFunctionType
X_AXIS = mybir.AxisListType.X

BS, H, W, C = 64, 64, 64, 128
N = H * W  # 4096 points per sample
NCORES = 8
SPB = BS // NCORES  # samples per core
NT = N // 128  # 32 tiles of 128 points
NG = 4  # tile groups of 8
GT = NT // NG  # tiles per group
EPS = 1e-5
ACLIP = 1.0 + 1e-7
LN_HALF = float(np.log(0.5))

# Engine running each transposed-group PSUM->SBUF copy ('v' = DVE,
# 'p' = GpSimd/Pool).
XBT_ENGINES = "vppp"
# Emission order: 'wave' = software-pipeline wavefront, 'sample' = natural.
EMIT_MODE = "sample"


# ---------------------------------------------------------------------------
# Tile drain patch: the walrus CoreV3 codegen in this container accepts only
# one sync-wait per CTRL (Drain) instruction, but Tile's final drain piles the
# whole global clock onto a single Drain. Split across chained SP drains.
def _patched_drain_and_barrier(self, tick_clock, wait_clock):
    nc = self.nc
    drain_inst = nc.sync.drain()
    wait_clock.add_sem_waits(
        drain_inst.ins, ScopedClock({None: tick_clock.global_clock})
    )
    si = drain_inst.ins.sync_info
    waits = list(si.on_wait or [])
    if len(waits) > 1:
        si.on_wait = waits[:1]
        for w in waits[1:]:
            d2 = nc.sync.drain()
            si2 = d2.ins.sync_info
            if si2 is None:
                d2.ins.sync_info = mybir.SyncInfo(on_wait=[w], on_update=[])
            else:
                si2.on_wait = [w]
    nc.all_engine_barrier()
    assert self.sems is not None
    popped = nc._tile_sem_poison_stack.pop()
    assert popped is self._sem_poison
    nc.clear_and_free_semaphores(list(self.sems.allocated().values()))
    nc.all_engine_barrier()


_orig_lower_ordered_insts = tile.TileContext._lower_ordered_insts
_wsplit_counter = [0]


def _patched_lower_ordered_insts(self, ordered):
    """Walrus here allows only one sync-wait per instruction; hoist extra
    waits onto same-engine NoOps inserted just before the instruction."""
    maxw = 1
    for insts in ordered.values():
        out = []
        for inst in insts:
            si = inst.sync_info
            waits = list(si.on_wait) if si is not None and si.on_wait else []
            if len(waits) > maxw:
                extra, keep = waits[:-maxw], waits[-maxw:]
                for i in range(0, len(extra), maxw):
                    _wsplit_counter[0] += 1
                    nop = mybir.InstNoOp(
                        name=f"wsplit-{_wsplit_counter[0]}",
                        engine=inst.engine,
                        ins=[],
                        outs=[],
                        sync_info=mybir.SyncInfo(
                            on_wait=extra[i : i + maxw], on_update=[]
                        ),
                    )
                    out.append(nop)
                si.on_wait = keep
            out.append(inst)
        insts[:] = out
    return _orig_lower_ordered_insts(self, ordered)


def _install_tile_patch():
    tile.TileContext._drain_and_barrier = _patched_drain_and_barrier
    tile.TileContext._lower_ordered_insts = _patched_lower_ordered_insts


# ---------------------------------------------------------------------------


def _sample_stages(nc, tc, pools, consts, x_view, y_view):
    """Return the list of per-stage emission closures for one sample."""
    (xpool, xbtpool, t1pool, ypool, chain, stats, psT, psY, misc) = pools
    (
        ident_h, ident_f, ones128, ones_row, signc, gamma_col, bm1, bln05, mask8,
    ) = consts

    # PSUM misc bank regions (f32 [128, 512] tile, allocated once):
    pa = misc[:, 0:32]  # alpha per point
    pv = misc[:, 32:33]  # var cross-partition sum
    bc = misc[:, 36:39]  # rn/mean0/i1p broadcast columns
    ss_ps = misc[0:1, 40:41]  # sum of S^2
    S_row = misc[0:1, 64:192]  # S as a row (transpose of S_col)
    S_rep = misc[0:8, 192:320]  # S row replicated on 8 partitions

    st = {}  # cross-stage tiles

    def s_load():
        xs = x_view.rearrange("(p t) c -> p t c", t=NT)
        xb = st["xb"] = xpool.tile([128, NT, C], f16, tag="xb", name="xb")
        for h in range(2):
            sl = slice(h * (NT // 2), (h + 1) * (NT // 2))
            nc.sync.dma_start(out=xb[:, sl, :], in_=xs[:, sl, :])

    def s_transpose(gs):
        def run():
            xb = st["xb"]
            if "xbt" not in st:
                st["xbt"] = xbtpool.tile([128, NT, C], f16, tag="xbt", name="xbt")
                st["Sp"] = stats.tile([128, NG], f32, tag="Sp", name="Sp")
            xbt, Sp = st["xbt"], st["Sp"]
            for g in gs:
                pt = psT.tile([128, GT, C], f16, tag="pt")
                for k in range(GT):
                    nc.tensor.transpose(pt[:, k, :], xb[:, GT * g + k, :], ident_h)
                eng = nc.vector if XBT_ENGINES[g] == "v" else nc.gpsimd
                eng.tensor_scalar(
                    out=xbt[:, GT * g : GT * (g + 1), :].rearrange(
                        "p a c -> p (a c)"
                    ),
                    in0=pt.rearrange("p a c -> p (a c)"),
                    scalar1=1.0,
                    scalar2=None,
                    op0=ALU.mult,
                    accum_out=Sp[:, g : g + 1],
                )

        return run

    def s_stats():
        ctx = tc.high_priority()
        ctx.__enter__()
        Sp = st["Sp"]
        S_col = st["S_col"] = stats.tile([128, 1], f32, tag="Scol", name="Scol")
        S01 = chain.tile([128, 1], f32, tag="S01")
        nc.vector.tensor_add(S01, Sp[:, 0:1], Sp[:, 1:2])
        S23 = chain.tile([128, 1], f32, tag="S23")
        nc.vector.tensor_add(S23, Sp[:, 2:3], Sp[:, 3:4])
        nc.vector.tensor_add(S_col, S01, S23)

        nc.tensor.transpose(S_row, S_col, ident_f)  # S as [1, 128] row
        nc.tensor.matmul(ss_ps, S_col, S_col, start=True, stop=True)
        # nls = -linner(S,S) = 2*S0^2 - ss
        S0 = S_col[0:1, 0:1]
        s0sq2 = chain.tile([1, 1], f32, tag="s0sq2")
        nc.vector.tensor_scalar(
            out=s0sq2, in0=S0, scalar1=S0, scalar2=2.0, op0=ALU.mult, op1=ALU.mult
        )
        nls = chain.tile([1, 1], f32, tag="nls")
        nc.vector.tensor_scalar(
            out=nls, in0=ss_ps, scalar1=-1.0, scalar2=s0sq2,
            op0=ALU.mult, op1=ALU.add,
        )
        sqn = chain.tile([1, 1], f32, tag="sqn")
        nc.scalar.activation(sqn, nls, ACTF.Sqrt)
        rn = chain.tile([1, 1], f32, tag="rn")  # 1/sqrt(nls)
        nc.vector.reciprocal(rn, sqn)
        mean0 = chain.tile([1, 1], f32, tag="mean0")
        nc.vector.tensor_scalar_mul(mean0, S0, rn)
        t1p = chain.tile([1, 1], f32, tag="t1p")
        nc.vector.tensor_scalar_add(t1p, mean0, 1.0)
        i1p = chain.tile([1, 1], f32, tag="i1p")  # 1/(1+mean0)
        nc.vector.reciprocal(i1p, t1p)
        # broadcast rn/mean0/i1p to all 128 partitions via [1,1] matmuls
        nc.tensor.matmul(bc[:, 0:1], ones_row, rn, start=True, stop=True)
        nc.tensor.matmul(bc[:, 1:2], ones_row, mean0, start=True, stop=True)
        nc.tensor.matmul(bc[:, 2:3], ones_row, i1p, start=True, stop=True)
        bcs = st["bcs"] = stats.tile([128, 3], f32, tag="bcs", name="bcs")
        nc.vector.tensor_copy(bcs, bc)
        rn_b = bcs[:, 0:1]

        # W column (f16): w = -S*rn except w[0] = +S0*rn
        Wb = st["Wb"] = stats.tile([128, 1], f16, tag="Wb", name="Wb")
        nc.vector.scalar_tensor_tensor(
            out=Wb, in0=S_col, scalar=rn_b, in1=signc, op0=ALU.mult, op1=ALU.mult
        )
        # mean row (f16) on 8 partitions -> blockdiag rhs8 = I8 (x) mean
        S_row_sb = stats.tile([1, C], f32, tag="Srow")
        nc.scalar.copy(S_row_sb, S_row)
        nc.tensor.matmul(
            S_rep, ones_row[0:1, 0:8], S_row_sb, start=True, stop=True
        )
        Mb8 = chain.tile([8, C], f16, tag="Mb8")
        nc.vector.tensor_scalar_mul(Mb8, S_rep, rn_b[0:8, 0:1])
        rhs8 = st["rhs8"] = stats.tile([8, GT, C], f16, tag="rhs8", name="rhs8")
        nc.vector.tensor_tensor(
            rhs8, mask8, Mb8[:, None, :].broadcast_to((8, GT, C)), ALU.mult
        )
        ctx.__exit__(None, None, None)

    def s_alpha():
        ctx = tc.high_priority()
        ctx.__enter__()
        xbt, Wb, xb = st["xbt"], st["Wb"], st["xb"]
        for t in range(NT):
            nc.tensor.matmul(
                pa[:, t : t + 1], xbt[:, t, :], Wb, start=True, stop=True
            )
        x0 = st["x0"] = chain.tile([128, NT], f32, tag="x0", name="x0")
        nc.gpsimd.tensor_copy(x0, xb[:, :, 0:1].rearrange("p t c -> p (t c)"))
        ctx.__exit__(None, None, None)

    def s_chain1():
        ctx = tc.high_priority()
        ctx.__enter__()
        bcs = st["bcs"]
        mean0_b, i1p_b = bcs[:, 1:2], bcs[:, 2:3]
        al = chain.tile([128, NT], f32, tag="al")
        nc.vector.tensor_scalar_max(al, pa, ACLIP)
        asq = chain.tile([128, NT], f32, tag="asq")
        nc.gpsimd.tensor_mul(asq, al, al)
        r = chain.tile([128, NT], f32, tag="r")  # sqrt(alpha^2-1)
        nc.scalar.activation(r, asq, ACTF.Sqrt, bias=bm1)
        rinv = st["rinv"] = chain.tile([128, NT], f32, tag="rinv", name="rinv")
        nc.vector.reciprocal(rinv, r)
        z = chain.tile([128, NT], f32, tag="z")
        nc.gpsimd.tensor_add(z, al, r)
        d = st["d"] = chain.tile([128, NT], f32, tag="d", name="d")  # arccosh(alpha)
        nc.scalar.activation(d, z, ACTF.Ln)
        negu0 = chain.tile([128, NT], f32, tag="negu0")  # alpha*mean0 - x0
        nc.vector.scalar_tensor_tensor(
            out=negu0, in0=al, scalar=mean0_b, in1=st["x0"],
            op0=ALU.mult, op1=ALU.subtract,
        )
        negC1 = st["negC1"] = chain.tile([128, NT], f32, tag="negC1", name="negC1")
        nc.vector.scalar_tensor_tensor(
            out=negC1, in0=negu0, scalar=i1p_b, in1=al,
            op0=ALU.mult, op1=ALU.subtract,
        )
        ctx.__exit__(None, None, None)

    def s_chain2():
        ctx = tc.high_priority()
        ctx.__enter__()
        d = st["d"]
        dsum = chain.tile([128, 1], f32, tag="dsum")
        nc.vector.tensor_reduce(dsum, d, axis=X_AXIS, op=ALU.add)
        nc.tensor.matmul(pv, ones128, dsum, start=True, stop=True)
        ve = chain.tile([128, 1], f32, tag="ve")
        nc.vector.tensor_scalar(
            out=ve, in0=pv, scalar1=1.0 / N, scalar2=EPS,
            op0=ALU.mult, op1=ALU.add,
        )
        rv = chain.tile([128, 1], f32, tag="rv")
        nc.vector.reciprocal(rv, ve)
        gg = chain.tile([128, 1], f32, tag="gg")
        nc.vector.tensor_mul(gg, gamma_col, rv)
        nu = chain.tile([128, NT], f32, tag="nu")
        nc.gpsimd.tensor_scalar_mul(nu, d, gg)
        Eh = st["Eh"] = chain.tile([128, NT], f32, tag="Eh", name="Eh")  # exp(nu)/2
        nc.scalar.activation(Eh, nu, ACTF.Exp, bias=bln05)
        Einvh = st["Einvh"] = chain.tile([128, NT], f32, tag="Einvh", name="Einvh")
        nc.scalar.activation(Einvh, nu, ACTF.Exp, scale=-1.0, bias=bln05)
        ctx.__exit__(None, None, None)

    def s_chain3():
        ctx = tc.high_priority()
        ctx.__enter__()
        Eh, Einvh = st["Eh"], st["Einvh"]
        sinh = chain.tile([128, NT], f32, tag="sinh")
        nc.gpsimd.tensor_sub(sinh, Eh, Einvh)
        cosh = st["cosh"] = chain.tile([128, NT], f32, tag="cosh", name="cosh")
        nc.gpsimd.tensor_add(cosh, Eh, Einvh)
        A = st["A"] = chain.tile([128, NT], f32, tag="A", name="A")
        nc.vector.tensor_mul(A, sinh, st["rinv"])
        negB16 = chain.tile([128, NT], f16, tag="negB16")
        nc.vector.tensor_mul(negB16, A, st["negC1"])
        # negB transposed per slab to [8, 128] base-0 stationaries
        negBT_ps = psT.tile([8, NG, C], f16, tag="nbt", bufs=1)
        for g in range(NG):
            nc.tensor.transpose(
                negBT_ps[:, g, :], negB16[:, GT * g : GT * (g + 1)], ident_h
            )
        negBT = st["negBT"] = stats.tile([8, NG, C], f16, tag="negBT", name="negBT")
        nc.vector.tensor_copy(
            negBT.rearrange("p a c -> p (a c)"),
            negBT_ps.rearrange("p a c -> p (a c)"),
        )
        ctx.__exit__(None, None, None)

    def s_y(gs):
        def run():
            xb, A, negBT, rhs8 = st["xb"], st["A"], st["negBT"], st["rhs8"]
            if "ysb" not in st:
                st["ysb"] = ypool.tile([128, NT, C], f16, tag="ysb", name="ysb")
            y_sb = st["ysb"]
            for g in gs:
                t1 = t1pool.tile([128, GT, C], f16, tag="t1")
                for k in range(GT):
                    t = GT * g + k
                    nc.vector.tensor_scalar_mul(
                        t1[:, k, :], xb[:, t, :], A[:, t : t + 1]
                    )
                yps = psY.tile([128, GT, C], f32, tag="yps")
                nc.tensor.matmul(
                    yps.rearrange("p a c -> p (a c)"),
                    ident_h,
                    t1.rearrange("p a c -> p (a c)"),
                    start=True,
                    stop=False,
                )
                nc.tensor.matmul(
                    yps.rearrange("p a c -> p (a c)"),
                    negBT[:, g, :],
                    rhs8.rearrange("p a c -> p (a c)"),
                    start=False,
                    stop=True,
                )
                nc.scalar.copy(
                    y_sb[:, GT * g : GT * (g + 1), :].rearrange(
                        "p a c -> p (a c)"
                    ),
                    yps.rearrange("p a c -> p (a c)"),
                )

        return run

    def s_store():
        y_sb, cosh = st["ysb"], st["cosh"]
        ys = y_view.rearrange("(p t) c -> p t c", t=NT)
        for h in range(2):
            sl = slice(h * (NT // 2), (h + 1) * (NT // 2))
            nc.gpsimd.tensor_copy(
                y_sb[:, sl, 0:1].rearrange("p t c -> p (t c)"), cosh[:, sl]
            )
            nc.sync.dma_start(out=ys[:, sl, :], in_=y_sb[:, sl, :])

    return [
        s_load,
        s_transpose([0, 1]),
        s_transpose([2, 3]),
        s_stats,
        s_alpha,
        s_chain1,
        s_chain2,
        s_chain3,
        s_y([0, 1]),
        s_y([2, 3]),
        s_store,
    ]


def build_program():
    _install_tile_patch()
    nc = bass.Bass("TRN2", debug=False)
    x_d = nc.dram_tensor("x", [SPB * N, C], f16, kind="ExternalInput").ap()
    g_d = nc.dram_tensor("gamma", [1], f32, kind="ExternalInput").ap()
    i_d = nc.dram_tensor("ident", [128, 128], f16, kind="ExternalInput").ap()
    y_d = nc.dram_tensor("y", [SPB * N, C], f16, kind="ExternalOutput").ap()

    with tile.TileContext(nc) as tc, ExitStack() as ctx:
        singles = ctx.enter_context(tc.tile_pool(name="singles", bufs=1))
        xpool = ctx.enter_context(tc.tile_pool(name="x", bufs=SPB))
        xbtpool = ctx.enter_context(tc.tile_pool(name="xbt", bufs=4))
        t1pool = ctx.enter_context(tc.tile_pool(name="t1", bufs=4))
        ypool = ctx.enter_context(tc.tile_pool(name="y", bufs=3))
        chain = ctx.enter_context(tc.tile_pool(name="chain", bufs=6))
        stats = ctx.enter_context(tc.tile_pool(name="stats", bufs=6))
        psT = ctx.enter_context(tc.tile_pool(name="psT", bufs=2, space="PSUM"))
        psY = ctx.enter_context(tc.tile_pool(name="psY", bufs=2, space="PSUM"))
        psM = ctx.enter_context(tc.tile_pool(name="psM", bufs=1, space="PSUM"))

        ident_h = singles.tile([128, 128], f16)
        nc.sync.dma_start(out=ident_h, in_=i_d)
        ident_f = singles.tile([128, 128], f32)
        nc.vector.tensor_copy(ident_f, ident_h)
        gamma_col = singles.tile([128, 1], f32)
        nc.sync.dma_start(out=gamma_col, in_=g_d.to_broadcast((128, 1)))
        ones128 = singles.tile([128, 128], f32)
        nc.vector.memset(ones128, 1.0)
        ones_row = singles.tile([1, 128], f32)
        nc.vector.memset(ones_row, 1.0)
        signc = singles.tile([128, 1], f32)
        nc.vector.memset(signc, -1.0)
        nc.vector.memset(signc[0:1, 0:1], 1.0)
        bm1 = singles.tile([128, 1], f32)
        nc.vector.memset(bm1, -1.0)
        bln05 = singles.tile([128, 1], f32)
        nc.vector.memset(bln05, LN_HALF)
        mask8 = singles.tile([8, GT, C], f16)
        nc.vector.memset(mask8.rearrange("p a c -> p (a c)"), 0.0)
        for k in range(GT):
            nc.vector.memset(mask8[k : k + 1, k, :], 1.0)
        misc = psM.tile([128, 512], f32, tag="misc")

        consts = (
            ident_h, ident_f, ones128, ones_row, signc, gamma_col, bm1, bln05,
            mask8,
        )
        pools = (xpool, xbtpool, t1pool, ypool, chain, stats, psT, psY, misc)

        samples = [
            _sample_stages(
                nc,
                tc,
                pools,
                consts,
                x_d[s * N : (s + 1) * N, :],
                y_d[s * N : (s + 1) * N, :],
            )
            for s in range(SPB)
        ]
        # all loads upfront (pure DMA, SP queue never blocks behind a store)
        for s in range(SPB):
            samples[s][0]()
        nstages = len(samples[0]) - 1
        if EMIT_MODE == "wave":
            for wave in range(SPB + nstages - 1):
                for s in range(SPB):
                    stg = wave - s
                    if 0 <= stg < nstages:
                        samples[s][stg + 1]()
        else:
            for s in range(SPB):
                for stg in range(nstages):
                    samples[s][stg + 1]()
    return nc


_PROGRAM = None


def _get_program():
    global _PROGRAM
    if _PROGRAM is None:
        _PROGRAM = build_program()
    return _PROGRAM


def _numpy_reference(x, beta, gamma):
    """Full-precision numpy fallback (general beta)."""
    CLAMP = 1e-8
    bs, h, w, c = x.shape
    x = x.reshape(bs, h * w, c).astype(np.float64)
    beta = beta.astype(np.float64)
    e0 = np.zeros(c)
    e0[0] = 1.0

    def linner(a, b):
        return (a * b).sum(-1, keepdims=True) - 2.0 * a[..., :1] * b[..., :1]

    m = x.mean(1, keepdims=True)
    mean = m / np.sqrt(np.clip(-linner(m, m), CLAMP, None))
    alpha = np.clip(-linner(mean, x), 1.0 + 1e-7, None)
    u = x - alpha * mean
    un = np.sqrt(np.clip(linner(u, u), CLAMP, None))
    x_T = np.arccosh(alpha) * u / un
    x_T = x_T - (x_T[..., :1] / (1.0 + mean[..., :1])) * (mean + e0)
    var = np.linalg.norm(x_T, axis=-1).mean(1)[:, None, None]
    x_T = x_T * (gamma.astype(np.float64) / (var + EPS))
    x_T = x_T + (linner(beta, x_T) / (1.0 + beta[0])) * (beta + e0)
    nu = np.sqrt(np.clip(linner(x_T, x_T), CLAMP, None))
    out = np.cosh(nu) * beta + np.sinh(nu) * x_T / nu
    return out.reshape(bs, h, w, c).astype(np.float32)


def kernel(x, beta, gamma):
    x = np.asarray(x, dtype=np.float32)
    beta = np.asarray(beta, dtype=np.float32)
    gamma = np.asarray(gamma, dtype=np.float32).reshape(1)

    e0 = np.zeros(C, np.float32)
    e0[0] = 1.0
    if not np.array_equal(beta, e0):
        return _numpy_reference(x, beta, gamma)

    from concourse.bass_utils import run_bass_kernel_spmd

    nc = _get_program()
    xr = np.ascontiguousarray(x.reshape(BS * N, C)).astype(np.float16)
    ident = np.eye(128, dtype=np.float16)
    in_maps = [
        {"x": xr[s * SPB * N : (s + 1) * SPB * N], "gamma": gamma, "ident": ident}
        for s in range(NCORES)
    ]
    res = run_bass_kernel_spmd(nc, in_maps, core_ids=list(range(NCORES)))
    y = np.concatenate([r["y"] for r in res.results], axis=0)
    return y.astype(np.float32).reshape(BS, H, W, C)


# revision 37
# speedup vs baseline: 1.0157x; 1.0157x over previous
"""Trainium2 Bass kernel for LorentzBatchNorm.

Math: for points x on the unit hyperboloid (linner(x,x) = -1) and the
normalized centroid `mean` (linner(mean,mean) = -1), the whole module
collapses per point to a rank-1 update:

  alpha = -linner(mean, x)            (one 128-dot per point)
  linner(u,u) = alpha^2 - 1           (u = x - alpha*mean; no 2nd reduction)
  d = arccosh(alpha) = ||x_T||        (Frechet var = mean of d)
  With beta = e0: transport to origin just zeroes channel 0, nu = g*d with
  g = gamma/(var+eps), and

  y[c] = A*x[c] - B*mean[c]  (c >= 1),   y[0] = cosh(nu)
  A = sinh(nu)/sqrt(alpha^2-1)
  B = A * (alpha + u0/(1+mean0)),  u0 = x0 - alpha*mean0

I/O strategy: x is converted to fp16 on the host (rounding ~2.4e-4, well
under the bf16 the previous version used internally) and y is produced as
fp16 on the device and upcast on the host — halving both DMA directions
makes the kernel DMA-bound at ~5.8us/sample (load 2.9 + store 2.9).

Per core (8 samples): points on partitions, channels on free dim,
[128, 32, 128] fp16 per sample; partition p holds points p*32..p*32+31.

Engines per sample (each under the 5.8us DMA budget):
 - PE: 32 per-tile transposes (alpha path), 32 alpha matvecs, and the y
   combine as PSUM matmuls: identity @ t1 (t1 = A*x) accumulated with a
   rank-8 blockdiag matmul negBT-slab @ (I8 (x) mean-row). Plus tiny
   transposes/broadcast matmuls.
 - DVE: t1 = A*x per tile (fp16 4x mode), some transposed-group PSUM->SBUF
   copies (centroid ridden along via accum_out), and the scalar chain.
 - ACT: the yps PSUM -> y_sb fp16 copies and the Sqrt/Ln/Exp chain ops.
 - Pool/GpSimd: remaining transposed-group copies, x0/cosh strided column
   ops, a few [128,32] chain ops.

The program is emitted as a software pipeline: each sample is split into
STAGES and emission is wavefront-ordered (stage = wave - sample), so every
engine's in-order instruction stream interleaves the in-flight samples and
the per-sample dependency chain (~13us deep) overlaps across samples.
"""

import sys

if "/opt/trn_rl_repo" not in sys.path:
    sys.path.insert(0, "/opt/trn_rl_repo")

from contextlib import ExitStack

import numpy as np

import concourse.bass as bass
import concourse.tile as tile
from concourse import mybir
from concourse.vector_clock import ScopedClock

f32 = mybir.dt.float32
f16 = mybir.dt.float16
ALU = mybir.AluOpType
ACTF = mybir.ActivationFunctionType
X_AXIS = mybir.AxisListType.X

BS, H, W, C = 64, 64, 64, 128
N = H * W  # 4096 points per sample
NCORES = 8
SPB = BS // NCORES  # samples per core
NT = N // 128  # 32 tiles of 128 points
NG = 4  # tile groups of 8
GT = NT // NG  # tiles per group
EPS = 1e-5
ACLIP = 1.0 + 1e-7
LN_HALF = float(np.log(0.5))

# Engine running each transposed-group PSUM->SBUF copy ('v' = DVE,
# 'p' = GpSimd/Pool).
XBT_ENGINES = "vppp"
# Emission order: 'wave' = software-pipeline wavefront, 'sample' = natural.
EMIT_MODE = "wave"


# ---------------------------------------------------------------------------
# Tile drain patch: the walrus CoreV3 codegen in this container accepts only
# one sync-wait per CTRL (Drain) instruction, but Tile's final drain piles the
# whole global clock onto a single Drain. Split across chained SP drains.
def _patched_drain_and_barrier(self, tick_clock, wait_clock):
    nc = self.nc
    drain_inst = nc.sync.drain()
    wait_clock.add_sem_waits(
        drain_inst.ins, ScopedClock({None: tick_clock.global_clock})
    )
    si = drain_inst.ins.sync_info
    waits = list(si.on_wait or [])
    if len(waits) > 1:
        si.on_wait = waits[:1]
        for w in waits[1:]:
            d2 = nc.sync.drain()
            si2 = d2.ins.sync_info
            if si2 is None:
                d2.ins.sync_info = mybir.SyncInfo(on_wait=[w], on_update=[])
            else:
                si2.on_wait = [w]
    nc.all_engine_barrier()
    assert self.sems is not None
    popped = nc._tile_sem_poison_stack.pop()
    assert popped is self._sem_poison
    nc.clear_and_free_semaphores(list(self.sems.allocated().values()))
    nc.all_engine_barrier()


_orig_lower_ordered_insts = tile.TileContext._lower_ordered_insts
_wsplit_counter = [0]


def _patched_lower_ordered_insts(self, ordered):
    """Walrus here allows only one sync-wait per instruction; hoist extra
    waits onto same-engine NoOps inserted just before the instruction."""
    maxw = 1
    for insts in ordered.values():
        out = []
        for inst in insts:
            si = inst.sync_info
            waits = list(si.on_wait) if si is not None and si.on_wait else []
            if len(waits) > maxw:
                extra, keep = waits[:-maxw], waits[-maxw:]
                for i in range(0, len(extra), maxw):
                    _wsplit_counter[0] += 1
                    nop = mybir.InstNoOp(
                        name=f"wsplit-{_wsplit_counter[0]}",
                        engine=inst.engine,
                        ins=[],
                        outs=[],
                        sync_info=mybir.SyncInfo(
                            on_wait=extra[i : i + maxw], on_update=[]
                        ),
                    )
                    out.append(nop)
                si.on_wait = keep
            out.append(inst)
        insts[:] = out
    return _orig_lower_ordered_insts(self, ordered)


def _install_tile_patch():
    tile.TileContext._drain_and_barrier = _patched_drain_and_barrier
    tile.TileContext._lower_ordered_insts = _patched_lower_ordered_insts


# ---------------------------------------------------------------------------


def _sample_stages(nc, tc, pools, consts, x_view, y_view):
    """Return the list of per-stage emission closures for one sample."""
    (xpool, xbtpool, t1pool, ypool, chain, stats, psT, psY, misc) = pools
    (
        ident_h, ident_f, ones128, ones_row, signc, gamma_col, bm1, bln05, mask8,
    ) = consts

    # PSUM misc bank regions (f32 [128, 512] tile, allocated once):
    pa = misc[:, 0:32]  # alpha per point
    pv = misc[:, 32:33]  # var cross-partition sum
    bc = misc[:, 36:39]  # rn/mean0/i1p broadcast columns
    ss_ps = misc[0:1, 40:41]  # sum of S^2
    S_row = misc[0:1, 64:192]  # S as a row (transpose of S_col)
    S_rep = misc[0:8, 192:320]  # S row replicated on 8 partitions

    st = {}  # cross-stage tiles

    def s_load():
        xs = x_view.rearrange("(p t) c -> p t c", t=NT)
        xb = st["xb"] = xpool.tile([128, NT, C], f16, tag="xb", name="xb")
        for h in range(2):
            sl = slice(h * (NT // 2), (h + 1) * (NT // 2))
            nc.sync.dma_start(out=xb[:, sl, :], in_=xs[:, sl, :])

    def s_transpose(gs):
        def run():
            xb = st["xb"]
            if "xbt" not in st:
                st["xbt"] = xbtpool.tile([128, NT, C], f16, tag="xbt", name="xbt")
                st["Sp"] = stats.tile([128, NG], f32, tag="Sp", name="Sp")
            xbt, Sp = st["xbt"], st["Sp"]
            for g in gs:
                pt = psT.tile([128, GT, C], f16, tag="pt")
                for k in range(GT):
                    nc.tensor.transpose(pt[:, k, :], xb[:, GT * g + k, :], ident_h)
                eng = nc.vector if XBT_ENGINES[g] == "v" else nc.gpsimd
                eng.tensor_scalar(
                    out=xbt[:, GT * g : GT * (g + 1), :].rearrange(
                        "p a c -> p (a c)"
                    ),
                    in0=pt.rearrange("p a c -> p (a c)"),
                    scalar1=1.0,
                    scalar2=None,
                    op0=ALU.mult,
                    accum_out=Sp[:, g : g + 1],
                )

        return run

    def s_stats():
        ctx = tc.high_priority()
        ctx.__enter__()
        Sp = st["Sp"]
        S_col = st["S_col"] = stats.tile([128, 1], f32, tag="Scol", name="Scol")
        S01 = chain.tile([128, 1], f32, tag="S01")
        nc.vector.tensor_add(S01, Sp[:, 0:1], Sp[:, 1:2])
        S23 = chain.tile([128, 1], f32, tag="S23")
        nc.vector.tensor_add(S23, Sp[:, 2:3], Sp[:, 3:4])
        nc.vector.tensor_add(S_col, S01, S23)

        nc.tensor.transpose(S_row, S_col, ident_f)  # S as [1, 128] row
        nc.tensor.matmul(ss_ps, S_col, S_col, start=True, stop=True)
        # nls = -linner(S,S) = 2*S0^2 - ss
        S0 = S_col[0:1, 0:1]
        s0sq2 = chain.tile([1, 1], f32, tag="s0sq2")
        nc.vector.tensor_scalar(
            out=s0sq2, in0=S0, scalar1=S0, scalar2=2.0, op0=ALU.mult, op1=ALU.mult
        )
        nls = chain.tile([1, 1], f32, tag="nls")
        nc.vector.tensor_scalar(
            out=nls, in0=ss_ps, scalar1=-1.0, scalar2=s0sq2,
            op0=ALU.mult, op1=ALU.add,
        )
        sqn = chain.tile([1, 1], f32, tag="sqn")
        nc.scalar.activation(sqn, nls, ACTF.Sqrt)
        rn = chain.tile([1, 1], f32, tag="rn")  # 1/sqrt(nls)
        nc.vector.reciprocal(rn, sqn)
        mean0 = chain.tile([1, 1], f32, tag="mean0")
        nc.vector.tensor_scalar_mul(mean0, S0, rn)
        t1p = chain.tile([1, 1], f32, tag="t1p")
        nc.vector.tensor_scalar_add(t1p, mean0, 1.0)
        i1p = chain.tile([1, 1], f32, tag="i1p")  # 1/(1+mean0)
        nc.vector.reciprocal(i1p, t1p)
        # broadcast rn/mean0/i1p to all 128 partitions via [1,1] matmuls
        nc.tensor.matmul(bc[:, 0:1], ones_row, rn, start=True, stop=True)
        nc.tensor.matmul(bc[:, 1:2], ones_row, mean0, start=True, stop=True)
        nc.tensor.matmul(bc[:, 2:3], ones_row, i1p, start=True, stop=True)
        bcs = st["bcs"] = stats.tile([128, 3], f32, tag="bcs", name="bcs")
        nc.vector.tensor_copy(bcs, bc)
        rn_b = bcs[:, 0:1]

        # W column (f16): w = -S*rn except w[0] = +S0*rn
        Wb = st["Wb"] = stats.tile([128, 1], f16, tag="Wb", name="Wb")
        nc.vector.scalar_tensor_tensor(
            out=Wb, in0=S_col, scalar=rn_b, in1=signc, op0=ALU.mult, op1=ALU.mult
        )
        # mean row (f16) on 8 partitions -> blockdiag rhs8 = I8 (x) mean
        S_row_sb = stats.tile([1, C], f32, tag="Srow")
        nc.scalar.copy(S_row_sb, S_row)
        nc.tensor.matmul(
            S_rep, ones_row[0:1, 0:8], S_row_sb, start=True, stop=True
        )
        Mb8 = chain.tile([8, C], f16, tag="Mb8")
        nc.vector.tensor_scalar_mul(Mb8, S_rep, rn_b[0:8, 0:1])
        rhs8 = st["rhs8"] = stats.tile([8, GT, C], f16, tag="rhs8", name="rhs8")
        nc.vector.tensor_tensor(
            rhs8, mask8, Mb8[:, None, :].broadcast_to((8, GT, C)), ALU.mult
        )
        ctx.__exit__(None, None, None)

    def s_alpha():
        ctx = tc.high_priority()
        ctx.__enter__()
        xbt, Wb, xb = st["xbt"], st["Wb"], st["xb"]
        for t in range(NT):
            nc.tensor.matmul(
                pa[:, t : t + 1], xbt[:, t, :], Wb, start=True, stop=True
            )
        x0 = st["x0"] = chain.tile([128, NT], f32, tag="x0", name="x0")
        nc.gpsimd.tensor_copy(x0, xb[:, :, 0:1].rearrange("p t c -> p (t c)"))
        ctx.__exit__(None, None, None)

    def s_chain1():
        ctx = tc.high_priority()
        ctx.__enter__()
        bcs = st["bcs"]
        mean0_b, i1p_b = bcs[:, 1:2], bcs[:, 2:3]
        al = chain.tile([128, NT], f32, tag="al")
        nc.vector.tensor_scalar_max(al, pa, ACLIP)
        asq = chain.tile([128, NT], f32, tag="asq")
        nc.gpsimd.tensor_mul(asq, al, al)
        r = chain.tile([128, NT], f32, tag="r")  # sqrt(alpha^2-1)
        nc.scalar.activation(r, asq, ACTF.Sqrt, bias=bm1)
        rinv = st["rinv"] = chain.tile([128, NT], f32, tag="rinv", name="rinv")
        nc.vector.reciprocal(rinv, r)
        z = chain.tile([128, NT], f32, tag="z")
        nc.gpsimd.tensor_add(z, al, r)
        d = st["d"] = chain.tile([128, NT], f32, tag="d", name="d")  # arccosh(alpha)
        nc.scalar.activation(d, z, ACTF.Ln)
        negu0 = chain.tile([128, NT], f32, tag="negu0")  # alpha*mean0 - x0
        nc.vector.scalar_tensor_tensor(
            out=negu0, in0=al, scalar=mean0_b, in1=st["x0"],
            op0=ALU.mult, op1=ALU.subtract,
        )
        negC1 = st["negC1"] = chain.tile([128, NT], f32, tag="negC1", name="negC1")
        nc.vector.scalar_tensor_tensor(
            out=negC1, in0=negu0, scalar=i1p_b, in1=al,
            op0=ALU.mult, op1=ALU.subtract,
        )
        ctx.__exit__(None, None, None)

    def s_chain2():
        ctx = tc.high_priority()
        ctx.__enter__()
        d = st["d"]
        dsum = chain.tile([128, 1], f32, tag="dsum")
        nc.vector.tensor_reduce(dsum, d, axis=X_AXIS, op=ALU.add)
        nc.tensor.matmul(pv, ones128, dsum, start=True, stop=True)
        ve = chain.tile([128, 1], f32, tag="ve")
        nc.vector.tensor_scalar(
            out=ve, in0=pv, scalar1=1.0 / N, scalar2=EPS,
            op0=ALU.mult, op1=ALU.add,
        )
        rv = chain.tile([128, 1], f32, tag="rv")
        nc.vector.reciprocal(rv, ve)
        gg = chain.tile([128, 1], f32, tag="gg")
        nc.vector.tensor_mul(gg, gamma_col, rv)
        nu = chain.tile([128, NT], f32, tag="nu")
        nc.gpsimd.tensor_scalar_mul(nu, d, gg)
        Eh = st["Eh"] = chain.tile([128, NT], f32, tag="Eh", name="Eh")  # exp(nu)/2
        nc.scalar.activation(Eh, nu, ACTF.Exp, bias=bln05)
        Einvh = st["Einvh"] = chain.tile([128, NT], f32, tag="Einvh", name="Einvh")
        nc.scalar.activation(Einvh, nu, ACTF.Exp, scale=-1.0, bias=bln05)
        ctx.__exit__(None, None, None)

    def s_chain3():
        ctx = tc.high_priority()
        ctx.__enter__()
        Eh, Einvh = st["Eh"], st["Einvh"]
        sinh = chain.tile([128, NT], f32, tag="sinh")
        nc.gpsimd.tensor_sub(sinh, Eh, Einvh)
        cosh = st["cosh"] = chain.tile([128, NT], f32, tag="cosh", name="cosh")
        nc.gpsimd.tensor_add(cosh, Eh, Einvh)
        A = st["A"] = chain.tile([128, NT], f32, tag="A", name="A")
        nc.vector.tensor_mul(A, sinh, st["rinv"])
        negB16 = chain.tile([128, NT], f16, tag="negB16")
        nc.vector.tensor_mul(negB16, A, st["negC1"])
        # negB transposed per slab to [8, 128] base-0 stationaries
        negBT_ps = psT.tile([8, NG, C], f16, tag="nbt", bufs=1)
        for g in range(NG):
            nc.tensor.transpose(
                negBT_ps[:, g, :], negB16[:, GT * g : GT * (g + 1)], ident_h
            )
        negBT = st["negBT"] = stats.tile([8, NG, C], f16, tag="negBT", name="negBT")
        nc.vector.tensor_copy(
            negBT.rearrange("p a c -> p (a c)"),
            negBT_ps.rearrange("p a c -> p (a c)"),
        )
        ctx.__exit__(None, None, None)

    def s_y(gs):
        def run():
            xb, A, negBT, rhs8 = st["xb"], st["A"], st["negBT"], st["rhs8"]
            if "ysb" not in st:
                st["ysb"] = ypool.tile([128, NT, C], f16, tag="ysb", name="ysb")
            y_sb = st["ysb"]
            for g in gs:
                t1 = t1pool.tile([128, GT, C], f16, tag="t1")
                for k in range(GT):
                    t = GT * g + k
                    nc.vector.tensor_scalar_mul(
                        t1[:, k, :], xb[:, t, :], A[:, t : t + 1]
                    )
                yps = psY.tile([128, GT, C], f32, tag="yps")
                nc.tensor.matmul(
                    yps.rearrange("p a c -> p (a c)"),
                    ident_h,
                    t1.rearrange("p a c -> p (a c)"),
                    start=True,
                    stop=False,
                )
                nc.tensor.matmul(
                    yps.rearrange("p a c -> p (a c)"),
                    negBT[:, g, :],
                    rhs8.rearrange("p a c -> p (a c)"),
                    start=False,
                    stop=True,
                )
                nc.scalar.copy(
                    y_sb[:, GT * g : GT * (g + 1), :].rearrange(
                        "p a c -> p (a c)"
                    ),
                    yps.rearrange("p a c -> p (a c)"),
                )

        return run

    def s_store():
        y_sb, cosh = st["ysb"], st["cosh"]
        ys = y_view.rearrange("(p t) c -> p t c", t=NT)
        for h in range(2):
            sl = slice(h * (NT // 2), (h + 1) * (NT // 2))
            nc.gpsimd.tensor_copy(
                y_sb[:, sl, 0:1].rearrange("p t c -> p (t c)"), cosh[:, sl]
            )
            nc.sync.dma_start(out=ys[:, sl, :], in_=y_sb[:, sl, :])

    return [
        s_load,
        s_transpose([0, 1]),
        s_transpose([2, 3]),
        s_stats,
        s_alpha,
        s_chain1,
        s_chain2,
        s_chain3,
        s_y([0, 1]),
        s_y([2, 3]),
        s_store,
    ]


def build_program():
    _install_tile_patch()
    nc = bass.Bass("TRN2", debug=False)
    x_d = nc.dram_tensor("x", [SPB * N, C], f16, kind="ExternalInput").ap()
    g_d = nc.dram_tensor("gamma", [1], f32, kind="ExternalInput").ap()
    i_d = nc.dram_tensor("ident", [128, 128], f16, kind="ExternalInput").ap()
    y_d = nc.dram_tensor("y", [SPB * N, C], f16, kind="ExternalOutput").ap()

    with tile.TileContext(nc) as tc, ExitStack() as ctx:
        singles = ctx.enter_context(tc.tile_pool(name="singles", bufs=1))
        xpool = ctx.enter_context(tc.tile_pool(name="x", bufs=SPB))
        xbtpool = ctx.enter_context(tc.tile_pool(name="xbt", bufs=4))
        t1pool = ctx.enter_context(tc.tile_pool(name="t1", bufs=4))
        ypool = ctx.enter_context(tc.tile_pool(name="y", bufs=3))
        chain = ctx.enter_context(tc.tile_pool(name="chain", bufs=6))
        stats = ctx.enter_context(tc.tile_pool(name="stats", bufs=6))
        psT = ctx.enter_context(tc.tile_pool(name="psT", bufs=2, space="PSUM"))
        psY = ctx.enter_context(tc.tile_pool(name="psY", bufs=2, space="PSUM"))
        psM = ctx.enter_context(tc.tile_pool(name="psM", bufs=1, space="PSUM"))

        ident_h = singles.tile([128, 128], f16)
        nc.sync.dma_start(out=ident_h, in_=i_d)
        ident_f = singles.tile([128, 128], f32)
        nc.vector.tensor_copy(ident_f, ident_h)
        gamma_col = singles.tile([128, 1], f32)
        nc.sync.dma_start(out=gamma_col, in_=g_d.to_broadcast((128, 1)))
        ones128 = singles.tile([128, 128], f32)
        nc.vector.memset(ones128, 1.0)
        ones_row = singles.tile([1, 128], f32)
        nc.vector.memset(ones_row, 1.0)
        signc = singles.tile([128, 1], f32)
        nc.vector.memset(signc, -1.0)
        nc.vector.memset(signc[0:1, 0:1], 1.0)
        bm1 = singles.tile([128, 1], f32)
        nc.vector.memset(bm1, -1.0)
        bln05 = singles.tile([128, 1], f32)
        nc.vector.memset(bln05, LN_HALF)
        mask8 = singles.tile([8, GT, C], f16)
        nc.vector.memset(mask8.rearrange("p a c -> p (a c)"), 0.0)
        for k in range(GT):
            nc.vector.memset(mask8[k : k + 1, k, :], 1.0)
        misc = psM.tile([128, 512], f32, tag="misc")

        consts = (
            ident_h, ident_f, ones128, ones_row, signc, gamma_col, bm1, bln05,
            mask8,
        )
        pools = (xpool, xbtpool, t1pool, ypool, chain, stats, psT, psY, misc)

        samples = [
            _sample_stages(
                nc,
                tc,
                pools,
                consts,
                x_d[s * N : (s + 1) * N, :],
                y_d[s * N : (s + 1) * N, :],
            )
            for s in range(SPB)
        ]
        # all loads upfront (pure DMA, SP queue never blocks behind a store)
        for s in range(SPB):
            samples[s][0]()
        nstages = len(samples[0]) - 1
        if EMIT_MODE == "wave":
            for wave in range(SPB + nstages - 1):
                for s in range(SPB):
                    stg = wave - s
                    if 0 <= stg < nstages:
                        samples[s][stg + 1]()
        else:
            for s in range(SPB):
                for stg in range(nstages):
                    samples[s][stg + 1]()
    return nc


_PROGRAM = None


def _get_program():
    global _PROGRAM
    if _PROGRAM is None:
        _PROGRAM = build_program()
    return _PROGRAM


def _numpy_reference(x, beta, gamma):
    """Full-precision numpy fallback (general beta)."""
    CLAMP = 1e-8
    bs, h, w, c = x.shape
    x = x.reshape(bs, h * w, c).astype(np.float64)
    beta = beta.astype(np.float64)
    e0 = np.zeros(c)
    e0[0] = 1.0

    def linner(a, b):
        return (a * b).sum(-1, keepdims=True) - 2.0 * a[..., :1] * b[..., :1]

    m = x.mean(1, keepdims=True)
    mean = m / np.sqrt(np.clip(-linner(m, m), CLAMP, None))
    alpha = np.clip(-linner(mean, x), 1.0 + 1e-7, None)
    u = x - alpha * mean
    un = np.sqrt(np.clip(linner(u, u), CLAMP, None))
    x_T = np.arccosh(alpha) * u / un
    x_T = x_T - (x_T[..., :1] / (1.0 + mean[..., :1])) * (mean + e0)
    var = np.linalg.norm(x_T, axis=-1).mean(1)[:, None, None]
    x_T = x_T * (gamma.astype(np.float64) / (var + EPS))
    x_T = x_T + (linner(beta, x_T) / (1.0 + beta[0])) * (beta + e0)
    nu = np.sqrt(np.clip(linner(x_T, x_T), CLAMP, None))
    out = np.cosh(nu) * beta + np.sinh(nu) * x_T / nu
    return out.reshape(bs, h, w, c).astype(np.float32)


def kernel(x, beta, gamma):
    x = np.asarray(x, dtype=np.float32)
    beta = np.asarray(beta, dtype=np.float32)
    gamma = np.asarray(gamma, dtype=np.float32).reshape(1)

    e0 = np.zeros(C, np.float32)
    e0[0] = 1.0
    if not np.array_equal(beta, e0):
        return _numpy_reference(x, beta, gamma)

    from concourse.bass_utils import run_bass_kernel_spmd

    nc = _get_program()
    xr = np.ascontiguousarray(x.reshape(BS * N, C)).astype(np.float16)
    ident = np.eye(128, dtype=np.float16)
    in_maps = [
        {"x": xr[s * SPB * N : (s + 1) * SPB * N], "gamma": gamma, "ident": ident}
        for s in range(NCORES)
    ]
    res = run_bass_kernel_spmd(nc, in_maps, core_ids=list(range(NCORES)))
    y = np.concatenate([r["y"] for r in res.results], axis=0)
    return y.astype(np.float32).reshape(BS, H, W, C)
